# revision 24
# baseline (speedup 1.0000x reference)
"""Causal self-attention Trainium2 kernel (8-core SPMD).

Problem: x[2,2048,1024], causal mask, Wqkv[3072,1024], Wo[1024,1024], fp32.
  qkv = x @ Wqkv.T ; per-head causal softmax attention ; out = attn @ Wo.T

Sharding (data + tensor parallel, per the head dimension):
  core c -> batch b = c // 4, heads {4g..4g+3} with g = c % 4.
  Each core computes Q,K,V for its 4 heads (512 qk cols + 256 v cols of the
  projection), runs causal attention for those heads, and multiplies by the
  matching 256 columns of Wo, producing a partial [2048, 1024] output (bf16).
  Host sums the 4 partials per batch (the tensor-parallel reduction).

Kernel structure (per core):
  - bf16 matmul operands (PE 1 cyc/row), fp32 PSUM accumulation.
  - Projection chunks ko-outer so PE tracks DMA arrival; input DMA is ordered
    (wqkT[ko] + xT[ko, s-chunk0]) pairs first so the first q-chunk's
    projection completes after ~2MB instead of ~5MB of input traffic.
  - Attention per q-chunk runs in two head-pair passes. Scores are computed
    TRANSPOSED (scoresT[k, q]); the two heads of a pair occupy disjoint
    partition halves of qkT (hp 0 / 64), so their QK^T matmuls are issued
    back-to-back and run CONCURRENTLY in the PE via row tiling
    (tile_position (0,0) / (64,0)).
  - Head-pair score blocks land in one 2-bank PSUM tile ([h_even | h_odd]
    512 cols each) so one ACTIVATE exps 1024 columns; the diagonal-straddler
    case exps both valid sub-ranges with ONE strided 3D-AP ACTIVATE.
  - AV is col-tiled: head_even's V[128,64] targets PE columns 0-63, head_odd
    columns 64-127, writing the two partition halves of one PSUM bank
    concurrently. A second col-tiled pair of ones[128,64] matmuls
    accumulates the softmax denominators for both heads into another bank,
    replicated across the same partition halves - so normalization is one
    PSUM copy + one reciprocal + two partition-aligned multiplies, with no
    cross-partition broadcast at all.
  - Shared-PSUM-bank accumulation discipline: the AV/denominator banks are
    DVE-memset to zero and every matmul into them uses start=False, so each
    element either accumulates onto the zero (has_written bit set) or
    overwrites it (bit clear) - correct in both cases and independent of
    the scheduler's matmul ordering.
  - Causality: strictly-upper blocks skipped; diagonal straddlers compute
    only the valid columns; the 128x128 diagonal sub-block is exp'd
    unmasked then multiplied by a binary mask tile (from the mask input).
  - AV/denominator matmuls are emitted lagging the exp pipeline by 2
    k-blocks so the PE never queues directly behind an in-flight ACTIVATE.
  - Output partials are cast to bf16 and DMA'd out (queues alternated);
    the final chunk's casts alternate Scalar/Vector engines since ACT is
    idle by then. The host accumulates partials in fp32.
"""

import numpy as np

S = 2048
D = 1024
DH = 64
B = 2
NCORES = 8
HPC = 4  # heads per core
QKC = 2 * HPC * DH  # 512 q+k projection columns per core
VC = HPC * DH  # 256 v columns per core
P = 128
KO = D // P  # 8 contraction tiles
NQ = S // 512  # 4 q-chunks of 512
NSC = S // P  # 16 s-chunks of 128

_cache = {}


def _np_compute_dt():
    import ml_dtypes

    return ml_dtypes.bfloat16


def _build():
    import concourse.bacc as bacc
    import concourse.mybir as mybir
    import concourse.tile as tile

    F32 = mybir.dt.float32
    CDT = mybir.dt.bfloat16
    EXP = mybir.ActivationFunctionType.Exp
    COPYF = mybir.ActivationFunctionType.Copy

    nc = bacc.Bacc()
    # xT pre-chunked on host: [4 s-chunks, D, 512]
    xT_d = nc.dram_tensor("xT", [NQ, D, 512], CDT, kind="ExternalInput")
    wqkT_d = nc.dram_tensor("wqkT", [D, QKC], CDT, kind="ExternalInput")
    wvT_d = nc.dram_tensor("wvT", [D, VC], CDT, kind="ExternalInput")
    woT_d = nc.dram_tensor("woT", [VC, D], CDT, kind="ExternalInput")
    maskT_d = nc.dram_tensor("maskT", [P, P], CDT, kind="ExternalInput")
    out_d = nc.dram_tensor("out", [S, D], CDT, kind="ExternalOutput")

    with tile.TileContext(nc) as tc:
        with (
            tc.tile_pool(name="persist", bufs=1) as persist,
            tc.tile_pool(name="sb_small", bufs=4) as sb_small,
            tc.tile_pool(name="sb_exp", bufs=12) as sb_exp,
            tc.tile_pool(name="sb_out", bufs=3) as sb_out,
            tc.tile_pool(name="pp_big", bufs=2, space="PSUM") as pp_big,
            tc.tile_pool(name="pp_av", bufs=2, space="PSUM") as pp_av,
            tc.tile_pool(name="pp_o", bufs=2, space="PSUM") as pp_o,
        ):
            xT_sb = persist.tile([P, KO, S], CDT, tag="xT")
            wqkT_sb = persist.tile([P, KO, QKC], CDT, tag="wqkT")
            wvT_sb = persist.tile([P, KO, VC], CDT, tag="wvT")
            woT_sb = persist.tile([P, 2, D], CDT, tag="woT")
            maskT_sb = persist.tile([P, P], CDT, tag="maskT")
            qkT_sb = persist.tile([P, 4, S], CDT, tag="qkT")
            v_sb = persist.tile([P, NSC, HPC, DH], CDT, tag="v")
            attn_sb = persist.tile([P, 2, S], CDT, tag="attn")
            ones64 = persist.tile([P, DH], CDT, tag="ones64")
            nc.vector.memset(ones64[:], 1.0)

            # PE warm-up: ~3us of tiny self-contained matmuls during the
            # input-DMA dead window, so the HAM clock-gate reaches 8/8
            # (2.4GHz) before the first projection chain - otherwise its
            # first ~3.4us run at the cold 1.2GHz rate. Sized to finish
            # before the first real matmul's input DMA lands, so it can
            # never delay real work.
            ps_warm = pp_o.tile([P, 512], F32, tag="o")
            for _ in range(28):
                nc.tensor.matmul(
                    ps_warm[0:DH, 0:DH],
                    ones64[:],
                    ones64[:],
                    start=True,
                    stop=True,
                    skip_group_check=True,
                )

            # --- input DMAs: (wqkT[ko], xT[ko, qb0]) pairs first so the
            # qc=0 projection chains complete after ~2MB of traffic; then
            # maskT + wvT (unblock attention/v-proj qc0), then the
            # remaining xT s-chunks, woT last (needed only by outproj). ---
            for ko in range(KO):
                e1, e2 = (nc.sync, nc.gpsimd) if ko % 2 == 0 else (nc.gpsimd, nc.sync)
                e1.dma_start(wqkT_sb[:, ko, :], wqkT_d[ko * P : (ko + 1) * P, :])
                e2.dma_start(
                    xT_sb[:, ko, 0:512], xT_d[0, ko * P : (ko + 1) * P, :]
                )
            nc.sync.dma_start(maskT_sb[:], maskT_d[:])
            for ko in range(KO):
                e1 = nc.sync if ko % 2 == 0 else nc.gpsimd
                e1.dma_start(wvT_sb[:, ko, :], wvT_d[ko * P : (ko + 1) * P, :])
            for qb in range(1, NQ):
                for ko in range(KO):
                    e1 = nc.sync if ko % 2 == 0 else nc.gpsimd
                    e1.dma_start(
                        xT_sb[:, ko, qb * 512 : (qb + 1) * 512],
                        xT_d[qb, ko * P : (ko + 1) * P, :],
                    )
            nc.gpsimd.dma_start(woT_sb[:], woT_d.rearrange("(ct p) e -> p ct e", p=P))

            def emit_outproj(qc, final=False):
                for si in range(4):
                    sc = qc * 4 + si
                    for en in range(2):
                        u = si * 2 + en
                        ps_o = pp_o.tile([P, 512], F32, tag="o")
                        for ct in range(2):
                            nc.tensor.matmul(
                                ps_o[:],
                                attn_sb[:, ct, sc * P : (sc + 1) * P],
                                woT_sb[:, ct, en * 512 : (en + 1) * 512],
                                start=(ct == 0),
                                stop=(ct == 1),
                                skip_group_check=True,
                            )
                        o_sb = sb_out.tile([P, 512], CDT, tag="osb")
                        if final and u % 2 == 0:
                            nc.scalar.activation(o_sb[:], ps_o[:], COPYF)
                        else:
                            nc.vector.tensor_copy(out=o_sb[:], in_=ps_o[:])
                        eng = nc.sync if u % 2 == 0 else nc.gpsimd
                        eng.dma_start(
                            out_d[sc * P : (sc + 1) * P, en * 512 : (en + 1) * 512],
                            o_sb[:],
                        )

            for qc in range(NQ):
                # --- qk projection chunk: pjQ holds all-heads Q (blocks
                # 0,1), pjK all-heads K (blocks 2,3); ko-outer so PE tracks
                # DMA arrival ---
                pjQ = pp_big.tile([P, 1024], F32, tag="big", name="pjQ")
                pjK = pp_big.tile([P, 1024], F32, tag="big", name="pjK")
                for ko in range(KO):
                    for mm in range(4):
                        slot = pjQ if mm < 2 else pjK
                        nc.tensor.matmul(
                            slot[:, (mm % 2) * 512 : (mm % 2 + 1) * 512],
                            wqkT_sb[:, ko, mm * P : (mm + 1) * P],
                            xT_sb[:, ko, qc * 512 : (qc + 1) * 512],
                            start=(ko == 0),
                            stop=(ko == KO - 1),
                            skip_group_check=True,
                        )
                nc.vector.tensor_copy(
                    out=qkT_sb[:, 0:2, qc * 512 : (qc + 1) * 512],
                    in_=pjQ.rearrange("p (a b) -> p a b", a=2),
                )
                nc.vector.tensor_copy(
                    out=qkT_sb[:, 2:4, qc * 512 : (qc + 1) * 512],
                    in_=pjK.rearrange("p (a b) -> p a b", a=2),
                )

                # --- v projection for s-chunks 4qc..4qc+3 (4 bank chains) ---
                pvA = pp_big.tile([P, 1024], F32, tag="big", name="pvA")
                pvB = pp_big.tile([P, 1024], F32, tag="big", name="pvB")
                for ko in range(KO):
                    for j in range(4):
                        slot = pvA if j < 2 else pvB
                        sc = 4 * qc + j
                        nc.tensor.matmul(
                            slot[:, (j % 2) * 512 : (j % 2) * 512 + VC],
                            xT_sb[:, ko, sc * P : (sc + 1) * P],
                            wvT_sb[:, ko, :],
                            start=(ko == 0),
                            stop=(ko == KO - 1),
                            skip_group_check=True,
                        )
                for half, slot in ((0, pvA), (1, pvB)):
                    nc.vector.tensor_copy(
                        out=v_sb[:, 4 * qc + 2 * half : 4 * qc + 2 * half + 2, :, :],
                        in_=slot.rearrange("p (a h d) -> p a h d", a=2, h=8)[:, :, 0:HPC, :],
                    )

                # --- attention for q-chunk qc: two head-pair passes ---
                nkb = 4 * qc + 4  # causal: k blocks 0 .. 4qc+3
                for p in range(2):  # pair p covers heads (2p, 2p+1)
                    h0, h1 = 2 * p, 2 * p + 1
                    # pair_ps: raw AV for h0 on partitions 0-63, h1 on
                    # 64-127 (one shared bank). den_ps: denominators for
                    # h0 / h1 replicated on the same partition halves.
                    pair_ps = pp_av.tile([P, 512], F32, tag="av", name="pair")
                    den_ps = pp_av.tile([P, 512], F32, tag="av", name="den")
                    # zero the shared banks; every matmul below uses
                    # start=False, so each element either accumulates onto
                    # the memset zero (has_written set) or overwrites the
                    # zero with its value (bit clear) - correct in both
                    # cases and independent of matmul execution order.
                    nc.vector.memset(pair_ps[:], 0.0)
                    nc.vector.memset(den_ps[:], 0.0)
                    pend = []

                    def flush_av(pend=pend, nkb=nkb, h0=h0, h1=h1,
                                 pair_ps=pair_ps, den_ps=den_ps):
                        exp2, off, kb = pend.pop(0)
                        last = kb == nkb - 1
                        # col-tiled AV pair: h0 -> PE cols/partitions 0-63,
                        # h1 -> 64-127, concurrent.
                        nc.tensor.matmul(
                            pair_ps[0:DH, off:512],
                            v_sb[:, kb, h0, :],
                            exp2[:, off:512],
                            start=False,
                            stop=last,
                            skip_group_check=True,
                        )
                        nc.tensor.matmul(
                            pair_ps[DH:P, off:512],
                            v_sb[:, kb, h1, :],
                            exp2[:, 512 + off : 1024],
                            start=False,
                            stop=last,
                            skip_group_check=True,
                        )
                        # col-tiled denominator pair (ones stationary)
                        nc.tensor.matmul(
                            den_ps[0:DH, off:512],
                            ones64[:],
                            exp2[:, off:512],
                            start=False,
                            stop=last,
                            skip_group_check=True,
                        )
                        nc.tensor.matmul(
                            den_ps[DH:P, off:512],
                            ones64[:],
                            exp2[:, 512 + off : 1024],
                            start=False,
                            stop=last,
                            skip_group_check=True,
                        )

                    for kb in range(nkb):
                        m = kb - 4 * qc  # >= 0 on diagonal straddlers
                        off = max(0, m) * P
                        ps2 = pp_big.tile([P, 1024], F32, tag="big", name="ps2")
                        exp2 = sb_exp.tile([P, 1024], CDT, tag="exp")
                        # row-tiled head-pair QK^T: h0 on partitions 0-63
                        # (tile (0,0)), h1 on 64-127 (tile (64,0)) - issued
                        # back-to-back so they run concurrently in the PE.
                        nc.tensor.matmul(
                            ps2[:, off:512],
                            qkT_sb[0:DH, 2 + p, kb * P : (kb + 1) * P],
                            qkT_sb[0:DH, p, qc * 512 + off : (qc + 1) * 512],
                            start=True,
                            stop=True,
                            skip_group_check=True,
                        )
                        nc.tensor.matmul(
                            ps2[:, 512 + off : 1024],
                            qkT_sb[DH:P, 2 + p, kb * P : (kb + 1) * P],
                            qkT_sb[DH:P, p, qc * 512 + off : (qc + 1) * 512],
                            start=True,
                            stop=True,
                            skip_group_check=True,
                        )
                        if off == 0:
                            nc.scalar.activation(exp2[:], ps2[:], EXP, scale=0.125)
                        else:
                            # one strided ACTIVATE over both heads' valid
                            # column ranges
                            nc.scalar.activation(
                                exp2.rearrange("p (a b) -> p a b", a=2)[:, :, off:512],
                                ps2.rearrange("p (a b) -> p a b", a=2)[:, :, off:512],
                                EXP,
                                scale=0.125,
                            )
                        if m >= 0:
                            nc.vector.tensor_mul(
                                out=exp2[:, off : off + P],
                                in0=exp2[:, off : off + P],
                                in1=maskT_sb[:],
                            )
                            nc.vector.tensor_mul(
                                out=exp2[:, 512 + off : 512 + off + P],
                                in0=exp2[:, 512 + off : 512 + off + P],
                                in1=maskT_sb[:],
                            )
                        pend.append((exp2, off, kb))
                        # lag AV by 2 k-blocks so the PE isn't queued
                        # directly behind an in-flight ACTIVATE
                        if len(pend) > 2:
                            flush_av()
                    while pend:
                        flush_av()

                    # normalize: one PSUM->SBUF copy of the denominators,
                    # one reciprocal, two partition-aligned multiplies.
                    den_sb = sb_small.tile([P, 512], F32, tag="den")
                    nc.vector.tensor_copy(out=den_sb[:], in_=den_ps[:])
                    recip_sb = sb_small.tile([P, 512], F32, tag="recip")
                    nc.vector.reciprocal_approx_fast(out=recip_sb[:], in_=den_sb[:])
                    for h in (h0, h1):
                        hp = (h % 2) * DH
                        nc.vector.tensor_mul(
                            out=attn_sb[hp : hp + DH, p, qc * 512 : (qc + 1) * 512],
                            in0=pair_ps[hp : hp + DH, :],
                            in1=recip_sb[hp : hp + DH, :],
                        )

                # --- deferred output projection (previous q chunk) ---
                if qc > 0:
                    emit_outproj(qc - 1)
            emit_outproj(NQ - 1, final=True)

    nc.compile()
    return nc


def _get_nc():
    if "nc" not in _cache:
        _cache["nc"] = _build()
    return _cache["nc"]


def _shard(x, mask, Wqkv, Wo):
    cdt = _np_compute_dt()
    in_maps = []
    # binary mask for the transposed 128x128 diagonal block:
    # valid (mask==0) -> 1.0, masked (-inf/large-negative) -> 0.0
    maskT = np.ascontiguousarray((mask[0, 0, :P, :P].T >= 0).astype(cdt))
    for c in range(NCORES):
        b = c // 4
        g = c % 4
        heads = [4 * g + i for i in range(HPC)]
        q_rows = np.concatenate([np.arange(h * DH, (h + 1) * DH) for h in heads])
        k_rows = D + q_rows
        v_rows = 2 * D + q_rows
        qk_rows = np.concatenate([q_rows, k_rows])
        xT = x[b].T.astype(cdt)  # [D, S]
        xT_chunks = np.ascontiguousarray(
            xT.reshape(D, NQ, 512).transpose(1, 0, 2)
        )  # [NQ, D, 512]
        in_maps.append(
            {
                "xT": xT_chunks,
                "wqkT": np.ascontiguousarray(Wqkv[qk_rows, :].T.astype(cdt)),
                "wvT": np.ascontiguousarray(Wqkv[v_rows, :].T.astype(cdt)),
                "woT": np.ascontiguousarray(Wo[:, q_rows].T.astype(cdt)),
                "maskT": maskT,
            }
        )
    return in_maps


def kernel(x, mask, Wqkv, Wo, _trace=False):
    from concourse.bass_utils import run_bass_kernel_spmd

    x = np.asarray(x, dtype=np.float32)
    mask = np.asarray(mask, dtype=np.float32)
    Wqkv = np.asarray(Wqkv, dtype=np.float32)
    Wo = np.asarray(Wo, dtype=np.float32)

    nc = _get_nc()
    in_maps = _shard(x, mask, Wqkv, Wo)
    res = run_bass_kernel_spmd(nc, in_maps, core_ids=list(range(NCORES)), trace=_trace)
    _cache["last_result"] = res

    out = np.zeros((B, S, D), dtype=np.float32)
    for c in range(NCORES):
        out[c // 4] += res.results[c]["out"].astype(np.float32)
    return out
